# revision 1
# baseline (speedup 1.0000x reference)
"""EvolveGCN-O layer on 8 Trainium2 NeuronCores (Bass/Tile, SPMD).

Math (reference):
    W' = LSTM_step(gcn_weight)                     # [128,128]
    deg[c] = sum_{e: col_e=c} ew_e + 1             # self-loop weight 1
    dinv = 1/sqrt(deg)
    out[c] = dinv[c] * sum_{e->c} ew_e * dinv[row_e] * (X @ W')[row_e]
           = dinv[c] * (sum_{e->c} ew_e * (dinv*X)[row_e]) @ W'   (aggregate-first)

Distribution: core k owns destination nodes [6272k, 6272k+6272) (nodes padded
50000 -> 50176 = 392 blocks of 128). All per-core arrays are rotated by 6272k
so every core runs the identical program on "its nodes first" coordinates.
Edges are bucketed by destination block on the host; each block's edge list is
split by source row (<25088 / >=25088, the int16 gather-table limit), padded to
a multiple of 128, and processed as 128-edge tiles:
  - z tables z0/z1 [25088,128] bf16 hold dinv[n]*X[n]  (phase B, DMA+DVE only)
  - dma_gather pulls 128 source rows per tile into SBUF (256B/row)
  - a one-hot selector (iota == col_rel) * ew, built by one DVE tensor_scalar,
    is matmul'd against the gathered tile, PSUM-accumulating per dest block:
        agg[f, d] += sum_e gather[e, f] * selw[e, d]
  - per block: out = (agg^T @ W') * dinv[col]  (the @W' also fixes the layout)
Degrees come from a host-packed [node, slot] table reduced on the DVE.
"""

import sys

sys.path.insert(0, "/opt/trn_rl_repo")

from contextlib import ExitStack

import numpy as np

import concourse.bacc as bacc
import concourse.tile as tile
from concourse import mybir
from concourse.bass_utils import run_bass_kernel_spmd

F32 = mybir.dt.float32
BF16 = mybir.dt.bfloat16
I16 = mybir.dt.int16
I32 = mybir.dt.int32
AF = mybir.ActivationFunctionType
ALU = mybir.AluOpType

N = 50000
E = 800000
D = 128
NCORE = 8
NP = 50176            # padded node count = 392 * 128
OWN = NP // NCORE     # 6272 nodes per core
NBLK = NP // 128      # 392 blocks total
NBO = OWN // 128      # 49 blocks owned per core
ZR = NP // 2          # 25088 rows per z table (int16 index limit)
ZTILES = ZR // 128    # 196
GRP = 7               # dest blocks per gather group
NGRP = NBO // GRP     # 7 groups

_prog_cache: dict = {}
last_exec_time_ns = None
last_results = None


def _build_program(T_L: int, T_H: int, DEG_PAD: int):
    TLH = T_L + T_H
    nidx_l = GRP * T_L * 128
    nidx_h = GRP * T_H * 128

    nc = bacc.Bacc("TRN2", target_bir_lowering=False, debug=False,
                   num_devices=NCORE)

    xp_d = nc.dram_tensor("Xp", [NP, D], F32, kind="ExternalInput")
    edeg_d = nc.dram_tensor("edeg", [128, NBLK * DEG_PAD], F32,
                            kind="ExternalInput")
    idxl_d = nc.dram_tensor("idxL", [128, NGRP * nidx_l // 16], I16,
                            kind="ExternalInput")
    idxh_d = nc.dram_tensor("idxH", [128, NGRP * nidx_h // 16], I16,
                            kind="ExternalInput")
    ewn_d = nc.dram_tensor("ewN", [128, NBO * TLH], F32, kind="ExternalInput")
    coln_d = nc.dram_tensor("colN", [128, NBO * TLH], F32,
                            kind="ExternalInput")
    gw_d = nc.dram_tensor("gw", [D, D], F32, kind="ExternalInput")
    wih_d = nc.dram_tensor("wih", [4 * D, D], F32, kind="ExternalInput")
    bih_d = nc.dram_tensor("bih", [1, 4 * D], F32, kind="ExternalInput")
    bhh_d = nc.dram_tensor("bhh", [1, 4 * D], F32, kind="ExternalInput")
    z0_d = nc.dram_tensor("z0", [ZR, D], BF16)
    z1_d = nc.dram_tensor("z1", [ZR, D], BF16)
    out_d = nc.dram_tensor("outp", [OWN, D], F32, kind="ExternalOutput")

    with tile.TileContext(nc) as tc, ExitStack() as ctx:
        res = ctx.enter_context(tc.tile_pool(name="res", bufs=1))

        # residents: iota row (f32), identity (f32), dinv, W'
        iota_i = res.tile([128, 128], I32)
        nc.gpsimd.iota(iota_i[:], pattern=[[1, 128]], base=0,
                       channel_multiplier=0)
        iota_f = res.tile([128, 128], F32)
        nc.vector.tensor_copy(iota_f[:], iota_i[:])
        iotac_i = res.tile([128, 1], I32)
        nc.gpsimd.iota(iotac_i[:], pattern=[[1, 1]], base=0,
                       channel_multiplier=1)
        iotac_f = res.tile([128, 1], F32)
        nc.vector.tensor_copy(iotac_f[:], iotac_i[:])
        ident = res.tile([128, 128], F32)
        nc.vector.tensor_scalar(ident[:], iota_f[:], iotac_f[:], None,
                                op0=ALU.is_equal)
        dinv = res.tile([128, NBLK], F32)
        wp = res.tile([128, 128], F32)

        # ---------------- LSTM weight evolution ----------------
        with tc.tile_pool(name="wpool", bufs=1) as wpl, \
             tc.tile_pool(name="wps", bufs=1, space="PSUM") as wps:
            g_sb = wpl.tile([128, 128], F32)
            nc.sync.dma_start(g_sb[:], gw_d[:])
            gt_ps = wps.tile([128, 128], F32, tag="tp")
            nc.tensor.transpose(gt_ps[:], g_sb[:], ident[:])
            gt_sb = wpl.tile([128, 128], F32)
            nc.vector.tensor_copy(gt_sb[:], gt_ps[:])

            wih_t = wpl.tile([128, 512], F32)
            for j in range(4):
                wtmp = wpl.tile([128, 128], F32, tag="wtmp")
                nc.sync.dma_start(wtmp[:], wih_d[128 * j:128 * (j + 1), :])
                tp = wps.tile([128, 128], F32, tag="tp")
                nc.tensor.transpose(tp[:], wtmp[:], ident[:])
                nc.vector.tensor_copy(wih_t[:, 128 * j:128 * (j + 1)], tp[:])

            bih_sb = wpl.tile([1, 512], F32)
            nc.sync.dma_start(bih_sb[:], bih_d[:])
            bhh_sb = wpl.tile([1, 512], F32)
            nc.sync.dma_start(bhh_sb[:], bhh_d[:])
            b_sb = wpl.tile([1, 512], F32)
            nc.vector.tensor_tensor(b_sb[:], bih_sb[:], bhh_sb[:],
                                    op=ALU.add)
            ones_r = wpl.tile([1, 128], F32)
            nc.vector.memset(ones_r[:], 1.0)

            gates = wps.tile([128, 512], F32, tag="gates")
            nc.tensor.matmul(gates[:], gt_sb[:], wih_t[:], start=True,
                             stop=False)
            nc.tensor.matmul(gates[:], ones_r[:], b_sb[:], start=False,
                             stop=True)

            sig_i = wpl.tile([128, 128], F32)
            nc.scalar.activation(sig_i[:], gates[:, 0:128], AF.Sigmoid)
            tanh_g = wpl.tile([128, 128], F32)
            nc.scalar.activation(tanh_g[:], gates[:, 256:384], AF.Tanh)
            c_sb = wpl.tile([128, 128], F32)
            nc.vector.tensor_tensor(c_sb[:], sig_i[:], tanh_g[:], op=ALU.mult)
            tanh_c = wpl.tile([128, 128], F32)
            nc.scalar.activation(tanh_c[:], c_sb[:], AF.Tanh)
            sig_o = wpl.tile([128, 128], F32)
            nc.scalar.activation(sig_o[:], gates[:, 384:512], AF.Sigmoid)
            nc.vector.tensor_tensor(wp[:], sig_o[:], tanh_c[:], op=ALU.mult)

        # ---------------- degrees -> dinv ----------------
        with tc.tile_pool(name="dpool", bufs=3) as dpl:
            CH = 56  # blocks per chunk; 392 = 7 * 56
            degt = res.tile([128, NBLK], F32)
            for ci in range(NBLK // CH):
                dt_ = dpl.tile([128, CH * DEG_PAD], F32, tag="dt")
                nc.sync.dma_start(
                    dt_[:], edeg_d[:, ci * CH * DEG_PAD:(ci + 1) * CH * DEG_PAD])
                nc.vector.reduce_sum(
                    degt[:, ci * CH:(ci + 1) * CH],
                    dt_[:].rearrange("p (t s) -> p t s", s=DEG_PAD),
                    axis=mybir.AxisListType.X)
            sqt = dpl.tile([128, NBLK], F32, tag="sq")
            nc.scalar.activation(sqt[:], degt[:], AF.Sqrt, bias=1.0)
            nc.vector.reciprocal(dinv[:], sqt[:])

        # ---------------- phase B: z = dinv * X (bf16 tables) ----------------
        GB = 4  # node tiles per batch; divides ZTILES=196 and NBLK-ZTILES=196
        with tc.tile_pool(name="bpool", bufs=3) as bpl:
            for bi in range(NBLK // GB):
                xb = bpl.tile([128, GB, D], F32, tag="xb")
                nc.sync.dma_start(
                    xb[:],
                    xp_d[bi * GB * 128:(bi + 1) * GB * 128, :]
                    .rearrange("(g p) d -> p g d", p=128))
                zb = bpl.tile([128, GB, D], BF16, tag="zb")
                for gi in range(GB):
                    t = bi * GB + gi
                    nc.vector.tensor_scalar(zb[:, gi, :], xb[:, gi, :],
                                            dinv[:, t:t + 1], None,
                                            op0=ALU.mult)
                t0 = bi * GB
                if t0 + GB <= ZTILES:
                    tgt, roff = z0_d, t0 * 128
                else:
                    tgt, roff = z1_d, (t0 - ZTILES) * 128
                nc.sync.dma_start(
                    tgt[roff:roff + GB * 128, :]
                    .rearrange("(g p) d -> p g d", p=128),
                    zb[:])

        # ---------------- phase C: gather + aggregate ----------------
        with tc.tile_pool(name="meta", bufs=1) as mpl, \
             tc.tile_pool(name="cpool", bufs=3) as cpl, \
             tc.tile_pool(name="gpool", bufs=2) as gpl, \
             tc.tile_pool(name="cps", bufs=2, space="PSUM") as cps:
            ew_sb = mpl.tile([128, NBO * TLH], F32)
            nc.sync.dma_start(ew_sb[:], ewn_d[:])
            col_sb = mpl.tile([128, NBO * TLH], F32)
            nc.sync.dma_start(col_sb[:], coln_d[:])

            for g in range(NGRP):
                ixl = cpl.tile([128, nidx_l // 16], I16, tag="ixl")
                nc.sync.dma_start(
                    ixl[:], idxl_d[:, g * nidx_l // 16:(g + 1) * nidx_l // 16])
                ixh = cpl.tile([128, nidx_h // 16], I16, tag="ixh")
                nc.sync.dma_start(
                    ixh[:], idxh_d[:, g * nidx_h // 16:(g + 1) * nidx_h // 16])
                gl = gpl.tile([128, GRP * T_L, D], BF16, tag="gl")
                nc.gpsimd.dma_gather(gl[:], z0_d[:], ixl[:], nidx_l, nidx_l, D,
                                     single_packet=False)
                gh = gpl.tile([128, GRP * T_H, D], BF16, tag="gh")
                nc.gpsimd.dma_gather(gh[:], z1_d[:], ixh[:], nidx_h, nidx_h, D,
                                     single_packet=False)

                for b in range(GRP):
                    lb = g * GRP + b
                    agg = cps.tile([128, 128], F32, tag="agg")
                    for j in range(TLH):
                        if j < T_L:
                            src = gl[:, b * T_L + j, :]
                        else:
                            src = gh[:, b * T_H + (j - T_L), :]
                        cidx = lb * TLH + j
                        selw = cpl.tile([128, 128], BF16, tag="selw")
                        nc.vector.tensor_scalar(
                            selw[:], iota_f[:], col_sb[:, cidx:cidx + 1],
                            ew_sb[:, cidx:cidx + 1],
                            op0=ALU.is_equal, op1=ALU.mult)
                        nc.tensor.matmul(agg[:], src, selw[:],
                                         start=(j == 0), stop=(j == TLH - 1))
                    aggt = cpl.tile([128, 128], F32, tag="aggt")
                    nc.scalar.copy(aggt[:], agg[:])
                    ops = cps.tile([128, 128], F32, tag="ops")
                    nc.tensor.matmul(ops[:], aggt[:], wp[:], start=True,
                                     stop=True)
                    osb = cpl.tile([128, 128], F32, tag="osb")
                    nc.scalar.activation(osb[:], ops[:], AF.Copy,
                                         scale=dinv[:, lb:lb + 1])
                    nc.sync.dma_start(out_d[lb * 128:(lb + 1) * 128, :],
                                      osb[:])

    nc.compile()
    return nc


def _wrap_idx(seg: np.ndarray) -> np.ndarray:
    """[n] int16 -> [16, n/16] gather wrap (idx i at [i%16, i//16])."""
    return seg.reshape(-1, 16).T


def _prepare(X, edge_index, edge_weight, gcn_weight, w_ih, w_hh, b_ih, b_hh):
    """Host-side sharding: returns (compiled program, per-core input maps)."""
    X = np.asarray(X, dtype=np.float32)
    ei = np.asarray(edge_index).astype(np.int64)
    ew_in = np.asarray(edge_weight, dtype=np.float32)
    gw = np.asarray(gcn_weight, dtype=np.float32)
    wih = np.asarray(w_ih, dtype=np.float32)
    bih = np.asarray(b_ih, dtype=np.float32).reshape(1, -1)
    bhh = np.asarray(b_hh, dtype=np.float32).reshape(1, -1)

    n = X.shape[0]
    assert n == N and ei.shape[1] == E

    # ---- degree slot table (random edges only; self-loop via +1 on device)
    indeg = np.bincount(ei[1], minlength=N)
    deg_pad = int(max(40, indeg.max()))
    order = np.argsort(ei[1], kind="stable")
    csort = ei[1][order]
    starts = np.concatenate([[0], np.cumsum(np.bincount(csort, minlength=N))])
    rank = np.arange(E) - starts[csort]
    ew_deg = np.zeros((NP, deg_pad), np.float32)
    ew_deg[csort, rank] = ew_in[order]

    # ---- full edge list with self-loops
    row = np.concatenate([ei[0], np.arange(N, dtype=np.int64)])
    col = np.concatenate([ei[1], np.arange(N, dtype=np.int64)])
    ewa = np.concatenate([ew_in, np.ones(N, np.float32)])
    owner = col // OWN

    # per-core bucketing (two passes: sizes, then fill)
    per_core = []
    need_l = need_h = 1
    for k in range(NCORE):
        m = owner == k
        rk = (row[m] - OWN * k) % NP
        ck = col[m] - OWN * k
        blk = ck // 128
        ish = rk >= ZR
        key = blk * 2 + ish
        cnt = np.bincount(key, minlength=NBO * 2)
        need_l = max(need_l, int(np.ceil(cnt[0::2].max() / 128)))
        need_h = max(need_h, int(np.ceil(cnt[1::2].max() / 128)))
        per_core.append((m, rk, ck, blk, ish, key))

    t_l, t_h = max(need_l, 10), max(need_h, 9)
    pkey = (t_l, t_h, deg_pad)
    if pkey not in _prog_cache:
        _prog_cache[pkey] = _build_program(t_l, t_h, deg_pad)
    nc = _prog_cache[pkey]

    tlh = t_l + t_h
    xpad = np.zeros((NP, D), np.float32)
    xpad[:N] = X

    in_maps = []
    for k in range(NCORE):
        m, rk, ck, blk, ish, key = per_core[k]
        so = np.argsort(key, kind="stable")
        ks = key[so]
        cnt = np.bincount(ks, minlength=NBO * 2)
        st = np.concatenate([[0], np.cumsum(cnt)])[:-1]
        r2 = np.arange(len(ks)) - st[ks]
        zi = np.where(ish, rk - ZR, rk).astype(np.int16)[so]
        crel = (ck % 128).astype(np.float32)[so]
        ewk = ewa[m][so]

        li = np.zeros(NBO * t_l * 128, np.int16)
        hi = np.zeros(NBO * t_h * 128, np.int16)
        ewp = np.zeros((NBO, tlh, 128), np.float32)
        clp = np.zeros((NBO, tlh, 128), np.float32)
        ml = (ks % 2) == 0
        bl, rl = ks[ml] // 2, r2[ml]
        li[bl * (t_l * 128) + rl] = zi[ml]
        ewp[bl, rl // 128, rl % 128] = ewk[ml]
        clp[bl, rl // 128, rl % 128] = crel[ml]
        mh = ~ml
        bh, rh = ks[mh] // 2, r2[mh]
        hi[bh * (t_h * 128) + rh] = zi[mh]
        ewp[bh, t_l + rh // 128, rh % 128] = ewk[mh]
        clp[bh, t_l + rh // 128, rh % 128] = crel[mh]

        nl, nh = GRP * t_l * 128, GRP * t_h * 128
        idxl = np.tile(np.concatenate(
            [_wrap_idx(li[g * nl:(g + 1) * nl]) for g in range(NGRP)],
            axis=1), (8, 1))
        idxh = np.tile(np.concatenate(
            [_wrap_idx(hi[g * nh:(g + 1) * nh]) for g in range(NGRP)],
            axis=1), (8, 1))

        edeg_k = np.roll(ew_deg, -OWN * k, axis=0) \
            .reshape(NBLK, 128, deg_pad).transpose(1, 0, 2) \
            .reshape(128, NBLK * deg_pad)

        in_maps.append({
            "Xp": np.ascontiguousarray(np.roll(xpad, -OWN * k, axis=0)),
            "edeg": np.ascontiguousarray(edeg_k),
            "idxL": np.ascontiguousarray(idxl),
            "idxH": np.ascontiguousarray(idxh),
            "ewN": np.ascontiguousarray(
                ewp.transpose(2, 0, 1).reshape(128, NBO * tlh)),
            "colN": np.ascontiguousarray(
                clp.transpose(2, 0, 1).reshape(128, NBO * tlh)),
            "gw": gw, "wih": wih, "bih": bih, "bhh": bhh,
        })

    return nc, in_maps


def kernel(X, edge_index, edge_weight, gcn_weight, w_ih, w_hh, b_ih, b_hh):
    global last_exec_time_ns, last_results
    nc, in_maps = _prepare(X, edge_index, edge_weight, gcn_weight, w_ih, w_hh,
                           b_ih, b_hh)
    res = run_bass_kernel_spmd(nc, in_maps, list(range(NCORE)))
    last_exec_time_ns = res.exec_time_ns
    last_results = res
    full = np.concatenate([res.results[k]["outp"] for k in range(NCORE)],
                          axis=0)
    return full[:N]



# revision 2
# speedup vs baseline: 13.0133x; 13.0133x over previous
"""EvolveGCN-O layer on 8 Trainium2 NeuronCores (Bass/Tile, SPMD).

Math (reference):
    W' = LSTM_step(gcn_weight)                     # [128,128]
    deg[c] = sum_{e: col_e=c} ew_e + 1             # self-loop weight 1
    dinv = 1/sqrt(deg)
    out[c] = dinv[c] * sum_{e->c} ew_e * dinv[row_e] * (X @ W')[row_e]
           = dinv[c] * (sum_{e->c} ew_e * (dinv*X)[row_e]) @ W'   (aggregate-first)

Distribution: core k owns destination nodes [6272k, 6272k+6272) (nodes padded
50000 -> 50176 = 392 blocks of 128). All per-core arrays are rotated by 6272k
so every core runs the identical program on "its nodes first" coordinates.
Edges are bucketed by destination block on the host; each block's edge list is
split by source row (<25088 / >=25088, the int16 gather-table limit), padded to
a multiple of 128, and processed as 128-edge tiles:
  - z tables z0/z1 [25088,128] bf16 hold dinv[n]*X[n]  (phase B, DMA+DVE only)
  - dma_gather pulls 128 source rows per tile into SBUF (256B/row)
  - a one-hot selector (iota == col_rel) * ew, built by one DVE tensor_scalar,
    is matmul'd against the gathered tile, PSUM-accumulating per dest block:
        agg[f, d] += sum_e gather[e, f] * selw[e, d]
  - per block: out = (agg^T @ W') * dinv[col]  (the @W' also fixes the layout)
Degrees come from a host-packed [node, slot] table reduced on the DVE.
"""

import sys

sys.path.insert(0, "/opt/trn_rl_repo")

from contextlib import ExitStack

import numpy as np

import concourse.bacc as bacc
import concourse.tile as tile
from concourse import mybir
from concourse.bass_utils import run_bass_kernel_spmd

F32 = mybir.dt.float32
BF16 = mybir.dt.bfloat16
I16 = mybir.dt.int16
I32 = mybir.dt.int32
AF = mybir.ActivationFunctionType
ALU = mybir.AluOpType

N = 50000
E = 800000
D = 128
NCORE = 8
NP = 50176            # padded node count = 392 * 128
OWN = NP // NCORE     # 6272 nodes per core
NBLK = NP // 128      # 392 blocks total
NBO = OWN // 128      # 49 blocks owned per core
ZR = NP // 2          # 25088 rows per z table (int16 index limit)
ZTILES = ZR // 128    # 196
GRP = 7               # dest blocks per gather group
NGRP = NBO // GRP     # 7 groups

_prog_cache: dict = {}
last_exec_time_ns = None
last_results = None


def _build_program(T_L: int, T_H: int, DEG_PAD: int):
    TLH = T_L + T_H
    nidx_l = GRP * T_L * 128
    nidx_h = GRP * T_H * 128

    nc = bacc.Bacc("TRN2", target_bir_lowering=False, debug=False,
                   num_devices=NCORE, num_swdge_queues=4)

    xp_d = nc.dram_tensor("Xp", [NP, D], F32, kind="ExternalInput")
    edeg_d = nc.dram_tensor("edeg", [128, NBLK * DEG_PAD], F32,
                            kind="ExternalInput")
    idxl_d = nc.dram_tensor("idxL", [128, NGRP * nidx_l // 16], I16,
                            kind="ExternalInput")
    idxh_d = nc.dram_tensor("idxH", [128, NGRP * nidx_h // 16], I16,
                            kind="ExternalInput")
    ewn_d = nc.dram_tensor("ewN", [128, NBO * TLH], F32, kind="ExternalInput")
    coln_d = nc.dram_tensor("colN", [128, NBO * TLH], F32,
                            kind="ExternalInput")
    gw_d = nc.dram_tensor("gw", [D, D], F32, kind="ExternalInput")
    wih_d = nc.dram_tensor("wih", [4 * D, D], F32, kind="ExternalInput")
    bih_d = nc.dram_tensor("bih", [1, 4 * D], F32, kind="ExternalInput")
    bhh_d = nc.dram_tensor("bhh", [1, 4 * D], F32, kind="ExternalInput")
    z0_d = nc.dram_tensor("z0", [ZR, D], BF16)
    z1_d = nc.dram_tensor("z1", [ZR, D], BF16)
    out_d = nc.dram_tensor("outp", [OWN, D], F32, kind="ExternalOutput")

    with tile.TileContext(nc) as tc, ExitStack() as ctx:
        res = ctx.enter_context(tc.tile_pool(name="res", bufs=1))

        # residents: iota row (f32), identity (f32), dinv, W'
        iota_i = res.tile([128, 128], I32)
        nc.gpsimd.iota(iota_i[:], pattern=[[1, 128]], base=0,
                       channel_multiplier=0)
        iota_f = res.tile([128, 128], F32)
        nc.vector.tensor_copy(iota_f[:], iota_i[:])
        iotac_i = res.tile([128, 1], I32)
        nc.gpsimd.iota(iotac_i[:], pattern=[[1, 1]], base=0,
                       channel_multiplier=1)
        iotac_f = res.tile([128, 1], F32)
        nc.vector.tensor_copy(iotac_f[:], iotac_i[:])
        ident = res.tile([128, 128], F32)
        nc.vector.tensor_scalar(ident[:], iota_f[:], iotac_f[:], None,
                                op0=ALU.is_equal)
        dinv = res.tile([128, NBLK], F32)
        wp = res.tile([128, 128], F32)

        # ---------------- LSTM weight evolution ----------------
        with tc.tile_pool(name="wpool", bufs=1) as wpl, \
             tc.tile_pool(name="wps", bufs=1, space="PSUM") as wps:
            g_sb = wpl.tile([128, 128], F32)
            nc.sync.dma_start(g_sb[:], gw_d[:])
            gt_ps = wps.tile([128, 128], F32, tag="tp")
            nc.tensor.transpose(gt_ps[:], g_sb[:], ident[:])
            gt_sb = wpl.tile([128, 128], F32)
            nc.vector.tensor_copy(gt_sb[:], gt_ps[:])

            wih_t = wpl.tile([128, 512], F32)
            for j in range(4):
                wtmp = wpl.tile([128, 128], F32, tag="wtmp")
                nc.sync.dma_start(wtmp[:], wih_d[128 * j:128 * (j + 1), :])
                tp = wps.tile([128, 128], F32, tag="tp")
                nc.tensor.transpose(tp[:], wtmp[:], ident[:])
                nc.vector.tensor_copy(wih_t[:, 128 * j:128 * (j + 1)], tp[:])

            bih_sb = wpl.tile([1, 512], F32)
            nc.sync.dma_start(bih_sb[:], bih_d[:])
            bhh_sb = wpl.tile([1, 512], F32)
            nc.sync.dma_start(bhh_sb[:], bhh_d[:])
            b_sb = wpl.tile([1, 512], F32)
            nc.vector.tensor_tensor(b_sb[:], bih_sb[:], bhh_sb[:],
                                    op=ALU.add)
            ones_r = wpl.tile([1, 128], F32)
            nc.vector.memset(ones_r[:], 1.0)

            gates = wps.tile([128, 512], F32, tag="gates")
            nc.tensor.matmul(gates[:], gt_sb[:], wih_t[:], start=True,
                             stop=False)
            nc.tensor.matmul(gates[:], ones_r[:], b_sb[:], start=False,
                             stop=True)

            sig_i = wpl.tile([128, 128], F32)
            nc.scalar.activation(sig_i[:], gates[:, 0:128], AF.Sigmoid)
            tanh_g = wpl.tile([128, 128], F32)
            nc.scalar.activation(tanh_g[:], gates[:, 256:384], AF.Tanh)
            c_sb = wpl.tile([128, 128], F32)
            nc.vector.tensor_tensor(c_sb[:], sig_i[:], tanh_g[:], op=ALU.mult)
            tanh_c = wpl.tile([128, 128], F32)
            nc.scalar.activation(tanh_c[:], c_sb[:], AF.Tanh)
            sig_o = wpl.tile([128, 128], F32)
            nc.scalar.activation(sig_o[:], gates[:, 384:512], AF.Sigmoid)
            nc.vector.tensor_tensor(wp[:], sig_o[:], tanh_c[:], op=ALU.mult)

        # ---------------- degrees -> dinv ----------------
        with tc.tile_pool(name="dpool", bufs=3) as dpl:
            CH = 56  # blocks per chunk; 392 = 7 * 56
            degt = res.tile([128, NBLK], F32)
            for ci in range(NBLK // CH):
                dt_ = dpl.tile([128, CH * DEG_PAD], F32, tag="dt")
                nc.sync.dma_start(
                    dt_[:], edeg_d[:, ci * CH * DEG_PAD:(ci + 1) * CH * DEG_PAD])
                nc.vector.reduce_sum(
                    degt[:, ci * CH:(ci + 1) * CH],
                    dt_[:].rearrange("p (t s) -> p t s", s=DEG_PAD),
                    axis=mybir.AxisListType.X)
            sqt = dpl.tile([128, NBLK], F32, tag="sq")
            nc.scalar.activation(sqt[:], degt[:], AF.Sqrt, bias=1.0)
            nc.vector.reciprocal(dinv[:], sqt[:])

        # ---------------- phase B: z = dinv * X (bf16 tables) ----------------
        GB = 4  # node tiles per batch; divides ZTILES=196 and NBLK-ZTILES=196
        with tc.tile_pool(name="bpool", bufs=3) as bpl:
            for bi in range(NBLK // GB):
                xb = bpl.tile([128, GB, D], F32, tag="xb")
                nc.sync.dma_start(
                    xb[:],
                    xp_d[bi * GB * 128:(bi + 1) * GB * 128, :]
                    .rearrange("(g p) d -> p g d", p=128))
                zb = bpl.tile([128, GB, D], BF16, tag="zb")
                for gi in range(GB):
                    t = bi * GB + gi
                    nc.vector.tensor_scalar(zb[:, gi, :], xb[:, gi, :],
                                            dinv[:, t:t + 1], None,
                                            op0=ALU.mult)
                t0 = bi * GB
                if t0 + GB <= ZTILES:
                    tgt, roff = z0_d, t0 * 128
                else:
                    tgt, roff = z1_d, (t0 - ZTILES) * 128
                nc.sync.dma_start(
                    tgt[roff:roff + GB * 128, :]
                    .rearrange("(g p) d -> p g d", p=128),
                    zb[:])

        # ---------------- phase C: gather + aggregate ----------------
        with tc.tile_pool(name="meta", bufs=1) as mpl, \
             tc.tile_pool(name="cpool", bufs=3) as cpl, \
             tc.tile_pool(name="gpool", bufs=2) as gpl, \
             tc.tile_pool(name="cps", bufs=2, space="PSUM") as cps:
            ew_sb = mpl.tile([128, NBO * TLH], F32)
            nc.sync.dma_start(ew_sb[:], ewn_d[:])
            col_sb = mpl.tile([128, NBO * TLH], F32)
            nc.sync.dma_start(col_sb[:], coln_d[:])

            qrr = [0]
            for g in range(NGRP):
                ixl = cpl.tile([128, nidx_l // 16], I16, tag="ixl")
                nc.sync.dma_start(
                    ixl[:], idxl_d[:, g * nidx_l // 16:(g + 1) * nidx_l // 16])
                ixh = cpl.tile([128, nidx_h // 16], I16, tag="ixh")
                nc.sync.dma_start(
                    ixh[:], idxh_d[:, g * nidx_h // 16:(g + 1) * nidx_h // 16])
                ixlh = {}
                gl = gpl.tile([128, GRP * T_L, D], BF16, tag="gl")
                gh = gpl.tile([128, GRP * T_H, D], BF16, tag="gh")
                # split each gather in two and round-robin the 4 SWDGE
                # queues so descriptor processing runs 4-wide
                ixlh[id(gl)], ixlh[id(gh)] = ixl, ixh
                for (tile_sb, tbl, nidx, ntl) in (
                        (gl, z0_d, nidx_l, GRP * T_L),
                        (gh, z1_d, nidx_h, GRP * T_H)):
                    h1 = (ntl // 2) * 128
                    h2 = nidx - h1
                    nc.gpsimd.dma_gather(
                        tile_sb[:, :ntl // 2, :], tbl[:],
                        ixlh[id(tile_sb)][:, :h1 // 16], h1, h1, D,
                        single_packet=False, queue_num=qrr[0] % 4)
                    qrr[0] += 1
                    nc.gpsimd.dma_gather(
                        tile_sb[:, ntl // 2:, :], tbl[:],
                        ixlh[id(tile_sb)][:, h1 // 16:], h2, h2, D,
                        single_packet=False, queue_num=qrr[0] % 4)
                    qrr[0] += 1
                qrr[0] += 1  # rotate start queue across groups

                for b in range(GRP):
                    lb = g * GRP + b
                    agg = cps.tile([128, 128], F32, tag="agg")
                    for j in range(TLH):
                        if j < T_L:
                            src = gl[:, b * T_L + j, :]
                        else:
                            src = gh[:, b * T_H + (j - T_L), :]
                        cidx = lb * TLH + j
                        selw = cpl.tile([128, 128], BF16, tag="selw")
                        nc.vector.tensor_scalar(
                            selw[:], iota_f[:], col_sb[:, cidx:cidx + 1],
                            ew_sb[:, cidx:cidx + 1],
                            op0=ALU.is_equal, op1=ALU.mult)
                        nc.tensor.matmul(agg[:], src, selw[:],
                                         start=(j == 0), stop=(j == TLH - 1))
                    aggt = cpl.tile([128, 128], F32, tag="aggt")
                    nc.scalar.copy(aggt[:], agg[:])
                    ops = cps.tile([128, 128], F32, tag="ops")
                    nc.tensor.matmul(ops[:], aggt[:], wp[:], start=True,
                                     stop=True)
                    osb = cpl.tile([128, 128], F32, tag="osb")
                    nc.scalar.activation(osb[:], ops[:], AF.Copy,
                                         scale=dinv[:, lb:lb + 1])
                    nc.sync.dma_start(out_d[lb * 128:(lb + 1) * 128, :],
                                      osb[:])

    nc.compile()
    return nc


def _wrap_idx(seg: np.ndarray) -> np.ndarray:
    """[n] int16 -> [16, n/16] gather wrap (idx i at [i%16, i//16])."""
    return seg.reshape(-1, 16).T


def _prepare(X, edge_index, edge_weight, gcn_weight, w_ih, w_hh, b_ih, b_hh):
    """Host-side sharding: returns (compiled program, per-core input maps)."""
    X = np.asarray(X, dtype=np.float32)
    ei = np.asarray(edge_index).astype(np.int64)
    ew_in = np.asarray(edge_weight, dtype=np.float32)
    gw = np.asarray(gcn_weight, dtype=np.float32)
    wih = np.asarray(w_ih, dtype=np.float32)
    bih = np.asarray(b_ih, dtype=np.float32).reshape(1, -1)
    bhh = np.asarray(b_hh, dtype=np.float32).reshape(1, -1)

    n = X.shape[0]
    assert n == N and ei.shape[1] == E

    # ---- degree slot table (random edges only; self-loop via +1 on device)
    indeg = np.bincount(ei[1], minlength=N)
    deg_pad = int(max(40, indeg.max()))
    order = np.argsort(ei[1], kind="stable")
    csort = ei[1][order]
    starts = np.concatenate([[0], np.cumsum(np.bincount(csort, minlength=N))])
    rank = np.arange(E) - starts[csort]
    ew_deg = np.zeros((NP, deg_pad), np.float32)
    ew_deg[csort, rank] = ew_in[order]

    # ---- full edge list with self-loops
    row = np.concatenate([ei[0], np.arange(N, dtype=np.int64)])
    col = np.concatenate([ei[1], np.arange(N, dtype=np.int64)])
    ewa = np.concatenate([ew_in, np.ones(N, np.float32)])
    owner = col // OWN

    # per-core bucketing (two passes: sizes, then fill)
    per_core = []
    need_l = need_h = 1
    for k in range(NCORE):
        m = owner == k
        rk = (row[m] - OWN * k) % NP
        ck = col[m] - OWN * k
        blk = ck // 128
        ish = rk >= ZR
        key = blk * 2 + ish
        cnt = np.bincount(key, minlength=NBO * 2)
        need_l = max(need_l, int(np.ceil(cnt[0::2].max() / 128)))
        need_h = max(need_h, int(np.ceil(cnt[1::2].max() / 128)))
        per_core.append((m, rk, ck, blk, ish, key))

    t_l, t_h = max(need_l, 10), max(need_h, 9)
    pkey = (t_l, t_h, deg_pad)
    if pkey not in _prog_cache:
        _prog_cache[pkey] = _build_program(t_l, t_h, deg_pad)
    nc = _prog_cache[pkey]

    tlh = t_l + t_h
    xpad = np.zeros((NP, D), np.float32)
    xpad[:N] = X

    in_maps = []
    for k in range(NCORE):
        m, rk, ck, blk, ish, key = per_core[k]
        so = np.argsort(key, kind="stable")
        ks = key[so]
        cnt = np.bincount(ks, minlength=NBO * 2)
        st = np.concatenate([[0], np.cumsum(cnt)])[:-1]
        r2 = np.arange(len(ks)) - st[ks]
        zi = np.where(ish, rk - ZR, rk).astype(np.int16)[so]
        crel = (ck % 128).astype(np.float32)[so]
        ewk = ewa[m][so]

        li = np.zeros(NBO * t_l * 128, np.int16)
        hi = np.zeros(NBO * t_h * 128, np.int16)
        ewp = np.zeros((NBO, tlh, 128), np.float32)
        clp = np.zeros((NBO, tlh, 128), np.float32)
        ml = (ks % 2) == 0
        bl, rl = ks[ml] // 2, r2[ml]
        li[bl * (t_l * 128) + rl] = zi[ml]
        ewp[bl, rl // 128, rl % 128] = ewk[ml]
        clp[bl, rl // 128, rl % 128] = crel[ml]
        mh = ~ml
        bh, rh = ks[mh] // 2, r2[mh]
        hi[bh * (t_h * 128) + rh] = zi[mh]
        ewp[bh, t_l + rh // 128, rh % 128] = ewk[mh]
        clp[bh, t_l + rh // 128, rh % 128] = crel[mh]

        nl, nh = GRP * t_l * 128, GRP * t_h * 128
        idxl = np.tile(np.concatenate(
            [_wrap_idx(li[g * nl:(g + 1) * nl]) for g in range(NGRP)],
            axis=1), (8, 1))
        idxh = np.tile(np.concatenate(
            [_wrap_idx(hi[g * nh:(g + 1) * nh]) for g in range(NGRP)],
            axis=1), (8, 1))

        edeg_k = np.roll(ew_deg, -OWN * k, axis=0) \
            .reshape(NBLK, 128, deg_pad).transpose(1, 0, 2) \
            .reshape(128, NBLK * deg_pad)

        in_maps.append({
            "Xp": np.ascontiguousarray(np.roll(xpad, -OWN * k, axis=0)),
            "edeg": np.ascontiguousarray(edeg_k),
            "idxL": np.ascontiguousarray(idxl),
            "idxH": np.ascontiguousarray(idxh),
            "ewN": np.ascontiguousarray(
                ewp.transpose(2, 0, 1).reshape(128, NBO * tlh)),
            "colN": np.ascontiguousarray(
                clp.transpose(2, 0, 1).reshape(128, NBO * tlh)),
            "gw": gw, "wih": wih, "bih": bih, "bhh": bhh,
        })

    return nc, in_maps


def kernel(X, edge_index, edge_weight, gcn_weight, w_ih, w_hh, b_ih, b_hh):
    global last_exec_time_ns, last_results
    nc, in_maps = _prepare(X, edge_index, edge_weight, gcn_weight, w_ih, w_hh,
                           b_ih, b_hh)
    res = run_bass_kernel_spmd(nc, in_maps, list(range(NCORE)))
    last_exec_time_ns = res.exec_time_ns
    last_results = res
    full = np.concatenate([res.results[k]["outp"] for k in range(NCORE)],
                          axis=0)
    return full[:N]

